# revision 30
# baseline (speedup 1.0000x reference)
"""Trainium2 Bass kernel for nn_DiscriminationLoss (segment_reduce).

Strategy (8 NeuronCores, pixel-sharded):
  - Each core gets 1/8 of the 4M pixels: pred slice [8, 524288] f32 and
    labels slice [524288] i32.
  - Pixels are tiled [128 partitions x F free]. For each free column t
    (a "block" of 128 pixels), a one-hot matrix oh[p, j] = (labels[p,t]
    == j+1), j in 0..31 is built on DVE (label 0 = background dropped,
    as in the reference). One-hot generation is batched over FC blocks
    per tensor_tensor(is_equal), j-major so all access patterns are
    dense step-1 16-bit (DVE 2x perf mode, ~4.4us per 256-block chunk).
  - pred is scaled by 2^14 and cast to fp16 on ScalarE (the scale rides
    the activation's free affine). fp16 keeps ~2^-11 per-element error;
    the final loss error lands ~1e-5. The host unscales.
  - Per block the PE contracts: psum[9, 32] += st[128, 9]^T @ oh[128, 32]
    (st = 8 scaled-fp16 channels | ones; the ones column yields counts).
    PSUM accumulates over all 4096 blocks per core.
  - GpSimd does nothing: its SBUF port is shared with the DVE and any
    long GpSimd op blocks the DVE one-hot stream (measured 3-4x stalls).
  - Each core emits [9, 32]. Host sums partials over cores (the psum
    step) and evaluates the tiny O(K^2) pairwise tail in f64.
"""

import sys
import functools

sys.path.insert(0, "/opt/trn_rl_repo")

import numpy as np

C = 8
K = 32
NCORES = 8
H = W = 2048
PTOT = H * W
PCORE = PTOT // NCORES  # 524288
SIGMA_DIS = 3.0
PRED_SCALE = float(2.0**14)

FG = 512   # free-dim length per DMA group (128*FG pixels per group)
FC = 256   # free-dim length per one-hot chunk (blocks per tensor_tensor)
QB = 8     # pixel-blocks batched per matmul (block-diagonal trick)
WARM_MMS = 40  # PE warmup matmuls (trip the HAM clock gate to 2.4 GHz)


def build_nc(pcore=PCORE, fg=FG, fc=FC, qb=QB, warm=WARM_MMS):
    import concourse.bacc as bacc
    import concourse.tile as tile
    import concourse.mybir as mybir
    from contextlib import ExitStack

    ftot = pcore // 128
    assert pcore % 128 == 0
    fg = min(fg, ftot)
    assert ftot % fg == 0 and fg % fc == 0
    # small leading groups prime the pipeline faster
    if ftot >= 8 * fg and fg >= 2 * qb and fg % 2 == 0:
        group_sizes = [fg // 2, fg // 2] + [fg] * (ftot // fg - 1)
    else:
        group_sizes = [fg] * (ftot // fg)
    assert sum(group_sizes) == ftot
    f32 = mybir.dt.float32
    bf16 = mybir.dt.bfloat16
    fp16 = mybir.dt.float16
    i32 = mybir.dt.int32

    nch = C + 1
    ones_col = C
    assert fc % qb == 0

    nc = bacc.Bacc(
        "TRN2", target_bir_lowering=False, debug=False, num_devices=NCORES
    )
    pred_ext = nc.dram_tensor("pred", [C, pcore], f32, kind="ExternalInput")
    lab_ext = nc.dram_tensor("labels", [pcore], i32, kind="ExternalInput")
    iota_ext = nc.dram_tensor("iotarep", [128, K * qb], bf16, kind="ExternalInput")
    # rows 0..nch*qb-1: results; row 96: warmup dump (keeps warm MMs live)
    out_ext = nc.dram_tensor("out_s", [128, K * qb], f32, kind="ExternalOutput")

    with tile.TileContext(nc) as tc, ExitStack() as ctx:
        const_pool = ctx.enter_context(tc.tile_pool(name="const", bufs=1))
        slab32_pool = ctx.enter_context(tc.tile_pool(name="slab32", bufs=3))
        slabh_pool = ctx.enter_context(tc.tile_pool(name="slabh", bufs=2))
        labb_pool = ctx.enter_context(tc.tile_pool(name="labb", bufs=3))
        oh_pool = ctx.enter_context(tc.tile_pool(name="oh", bufs=6))
        psum_pool = ctx.enter_context(tc.tile_pool(name="psum", bufs=1, space="PSUM"))
        out_pool = ctx.enter_context(tc.tile_pool(name="outp", bufs=1))

        iota_t = const_pool.tile([128, K * qb], bf16)
        nc.sync.dma_start(iota_t[:], iota_ext[:])

        psum_full = psum_pool.tile([128, K * qb], f32)
        psum_t = psum_full[: nch * qb, :]

        # PE warmup: ~5us of dense matmuls so the HAM clock gate opens
        # (otherwise every matmul runs at the cold 1.2 GHz rate).
        warm_ps = psum_pool.tile([128, 256], f32)
        if warm:
            for w in range(warm):
                nc.tensor.matmul(
                    warm_ps[:],
                    iota_t[:, :128],
                    iota_t[:, : K * qb],
                    start=(w == 0),
                    stop=(w == warm - 1),
                )

        nblocks = ftot
        blk = 0
        goff = 0
        for fgg in group_sizes:
            gpx = 128 * fgg
            poff = 128 * goff
            slab32 = slab32_pool.tile([128, C * fg], f32, tag="slab32")
            s32 = slab32[:, : C * fgg]
            nc.sync.dma_start(
                s32.rearrange("p (c f) -> p c f", c=C),
                pred_ext[:, poff : poff + gpx].rearrange("c (p f) -> p c f", p=128),
            )
            # slabh layout: [p, (tg, c, b)], col = tg*(nch*qb) + c*qb + b —
            # each tg-group's stationary [128, nch*qb] is a contiguous slice.
            slabh = slabh_pool.tile([128, nch * fg], fp16, tag="slabh")
            slabh_r = slabh[:, : nch * fgg].rearrange(
                "p (tg c b) -> p tg c b", c=nch, b=qb
            )  # [128, fgg/qb, nch, qb]
            slab32_r = s32.rearrange(
                "p (c tg b) -> p tg c b", c=C, b=qb
            )  # in natural (c, t) layout: t = tg*qb + b
            # scaled fp16 cast on ScalarE: out = Copy(in * 2^14)
            nc.scalar.activation(
                slabh_r[:, :, :C, :],
                slab32_r,
                mybir.ActivationFunctionType.Copy,
                scale=PRED_SCALE,
            )
            # ones column via ACT: Copy(0*x + 1) = 1.0 (keeps DVE free)
            nc.scalar.activation(
                slabh_r[:, :, ones_col, :],
                s32[:, :fgg],
                mybir.ActivationFunctionType.Copy,
                bias=1.0,
                scale=0.0,
            )

            # labels arrive as bf16 via SWDGE cast-DMA (no DVE copy needed)
            lbt = labb_pool.tile([128, fg], bf16, tag="labb")
            nc.gpsimd.dma_start(
                lbt[:, :fgg],
                lab_ext[poff : poff + gpx].rearrange("(p f) -> p f", p=128),
            )

            fcg = min(fc, fgg)
            for ci in range(fgg // fcg):
                # oh layout: [p, (tg, j, b)] — each tg-group's moving
                # operand [128, K*qb] is a contiguous slice.
                oh = oh_pool.tile([128, K * fc], fp16, tag="oh")
                oh_r = oh[:, : K * fcg].rearrange(
                    "p (tg j b) -> p tg j b", j=K, b=qb
                )  # [128, fcg/qb, K, qb]
                in0 = (
                    lbt[:, ci * fcg : (ci + 1) * fcg]
                    .rearrange("p (tg b) -> p tg b", b=qb)
                    .unsqueeze(2)
                    .broadcast_to([128, fcg // qb, K, qb])
                )
                in1 = (
                    iota_t[:]
                    .rearrange("p (j b) -> p j b", b=qb)
                    .unsqueeze(1)
                    .broadcast_to([128, fcg // qb, K, qb])
                )
                nc.vector.tensor_tensor(oh_r, in0, in1, mybir.AluOpType.is_equal)
                for tg in range(fcg // qb):
                    tg_abs = (ci * fcg) // qb + tg
                    nc.tensor.matmul(
                        psum_t[:],
                        slabh[:, tg_abs * nch * qb : (tg_abs + 1) * nch * qb],
                        oh[:, tg * K * qb : (tg + 1) * K * qb],
                        start=(blk == 0),
                        stop=(blk == nblocks - qb),
                    )
                    blk += qb
            goff += fgg

        outt = out_pool.tile([128, K * qb], f32)
        nc.vector.memset(outt[:], 0.0)
        nc.vector.tensor_copy(outt[: nch * qb, :], psum_t[:])
        if warm:
            nc.vector.tensor_copy(outt[96:97, :], warm_ps[96:97, : K * qb])
        nc.sync.dma_start(out_ext[:], outt[:])
    nc.compile()
    return nc


def make_iota_np(qb=QB):
    import ml_dtypes

    # value j+1 at [p, j*qb + b]
    v = np.repeat(np.arange(1, K + 1, dtype=np.float32), qb)
    return np.broadcast_to(v, (128, K * qb)).astype(ml_dtypes.bfloat16)


@functools.lru_cache(maxsize=1)
def _get_program():
    return build_nc()


def make_in_maps(pred_flat, labels_flat):
    iota_np = make_iota_np()
    in_maps = []
    for i in range(NCORES):
        sl = slice(i * PCORE, (i + 1) * PCORE)
        in_maps.append(
            {
                "pred": np.ascontiguousarray(pred_flat[:, sl]),
                "labels": np.ascontiguousarray(labels_flat[sl]),
                "iotarep": iota_np,
            }
        )
    return in_maps


def finish_host(parts, num_kernel, qb=QB):
    """parts: per-core [9*qb+1, K*qb] partials. Tiny O(K^2) tail in f64."""
    nch = C + 1
    total = np.sum([p.astype(np.float64) for p in parts], axis=0)
    r = total[: nch * qb, :].reshape(nch, qb, K, qb)
    total = r[:, np.arange(qb), :, np.arange(qb)].sum(axis=0)  # [nch, K]
    S = total[:C, :] / PRED_SCALE  # [8, 32]
    N = total[C, :]  # [32]
    A = N * np.sum(S * S, axis=0)  # [32]
    kk = int(num_kernel)
    A = A[:kk]
    pair = A[:, None] + A[None, :]
    Dm = np.maximum(SIGMA_DIS - np.sqrt(pair), 0.0)
    term = np.log(Dm * Dm + 1.0)
    L = float(np.sum(np.triu(term, k=1)))
    L *= (kk - 1) / kk
    return np.float32(L)


_last_results = None


def kernel(pred_similarities, regions_mask, kernel_labels, num_kernel, **kw):
    global _last_results
    from concourse.bass_utils import run_bass_kernel_spmd

    pred_flat = np.asarray(pred_similarities, dtype=np.float32).reshape(C, PTOT)
    labels_flat = np.asarray(kernel_labels, dtype=np.int32).reshape(PTOT)

    nc = _get_program()
    in_maps = make_in_maps(pred_flat, labels_flat)
    res = run_bass_kernel_spmd(nc, in_maps, list(range(NCORES)))
    _last_results = res
    parts = [res.results[i]["out_s"] for i in range(NCORES)]
    return finish_host(parts, num_kernel)


# revision 31
# speedup vs baseline: 1.0778x; 1.0778x over previous
"""Trainium2 Bass kernel for nn_DiscriminationLoss (segment_reduce).

Measured: ~88-96 us HW exec on 8 cores, rel err ~2e-5 vs the f32
reference (DVE one-hot spine ~70 us + ~20 us NEFF init/drain overhead;
HBM roofline for the 18 MiB/core of traffic is ~52 us).

Strategy (8 NeuronCores, pixel-sharded):
  - Each core gets 1/8 of the 4M pixels: pred slice [8, 524288] f32 and
    labels slice [524288] i32.
  - Pixels are tiled [128 partitions x F free]. For each free column t
    (a "block" of 128 pixels), a one-hot matrix oh[p, j] = (labels[p,t]
    == j+1), j in 0..31 is built on DVE (label 0 = background dropped,
    as in the reference). One-hot generation is batched over FC blocks
    per tensor_tensor(is_equal); all access patterns are dense step-1
    16-bit, which engages the DVE 2x perf mode (~4.4us per 256-block
    chunk — this stream is the kernel's critical path).
  - pred is scaled by 2^14 and cast to fp16 on ScalarE (the scale rides
    the activation's free affine; the ones column for counts is written
    by a second activation with scale=0, bias=1). fp16 keeps ~2^-11
    per-element error; the final loss error lands ~2e-5. Host unscales.
  - The PE contracts QB=8 blocks per matmul (block-diagonal batching to
    amortize the per-instruction floor):
      psum[72, 256] += st[128, 72]^T @ oh[128, 256]
    where st packs 8 blocks' [8 fp16 channels | ones] side by side and
    oh packs their one-hots; only the 8 diagonal [9, 32] sub-blocks of
    the product are meaningful and the host extracts them. PSUM
    accumulates over all 512 matmuls per core. A short warmup burst of
    dense matmuls first trips the PE HAM clock gate to 2.4 GHz
    (otherwise every matmul pays the cold 1.2 GHz rate — measured 2x).
  - Layouts are permuted at the producer side (ACT cast and DVE one-hot
    write through strided APs) so every matmul operand is a contiguous
    single-free-dim slice, which walrus requires.
  - GpSimd only issues the labels' int32->bf16 cast-DMA (SWDGE); it
    must not run streaming SBUF ops: its SBUF port is shared with the
    DVE and any long GpSimd op blocks the one-hot stream (measured
    3-4x stalls on the TT when GpSimd streamed concurrently).
  - Each core emits [128, 256] (PSUM readout + warmup dump row). Host
    sums partials over cores (the "psum" step of the sharding hint) and
    evaluates the tiny O(K^2) pairwise tail in f64.
"""

import sys
import functools

sys.path.insert(0, "/opt/trn_rl_repo")

import numpy as np

C = 8
K = 32
NCORES = 8
H = W = 2048
PTOT = H * W
PCORE = PTOT // NCORES  # 524288
SIGMA_DIS = 3.0
PRED_SCALE = float(2.0**14)

FG = 512   # free-dim length per DMA group (128*FG pixels per group)
FC = 256   # free-dim length per one-hot chunk (blocks per tensor_tensor)
QB = 8     # pixel-blocks batched per matmul (block-diagonal trick)
WARM_MMS = 40  # PE warmup matmuls (trip the HAM clock gate to 2.4 GHz)


def build_nc(pcore=PCORE, fg=FG, fc=FC, qb=QB, warm=WARM_MMS):
    import concourse.bacc as bacc
    import concourse.tile as tile
    import concourse.mybir as mybir
    from contextlib import ExitStack

    ftot = pcore // 128
    assert pcore % 128 == 0
    fg = min(fg, ftot)
    assert ftot % fg == 0 and fg % fc == 0
    # small leading groups prime the pipeline faster
    if ftot >= 8 * fg and fg >= 2 * qb and fg % 2 == 0:
        group_sizes = [fg // 2, fg // 2] + [fg] * (ftot // fg - 1)
    else:
        group_sizes = [fg] * (ftot // fg)
    assert sum(group_sizes) == ftot
    f32 = mybir.dt.float32
    bf16 = mybir.dt.bfloat16
    fp16 = mybir.dt.float16
    i32 = mybir.dt.int32

    nch = C + 1
    ones_col = C
    assert fc % qb == 0

    nc = bacc.Bacc(
        "TRN2", target_bir_lowering=False, debug=False, num_devices=NCORES
    )
    pred_ext = nc.dram_tensor("pred", [C, pcore], f32, kind="ExternalInput")
    lab_ext = nc.dram_tensor("labels", [pcore], i32, kind="ExternalInput")
    iota_ext = nc.dram_tensor("iotarep", [128, K * qb], bf16, kind="ExternalInput")
    # rows 0..nch*qb-1: results; row 96: warmup dump (keeps warm MMs live)
    out_ext = nc.dram_tensor("out_s", [128, K * qb], f32, kind="ExternalOutput")

    with tile.TileContext(nc) as tc, ExitStack() as ctx:
        const_pool = ctx.enter_context(tc.tile_pool(name="const", bufs=1))
        slab32_pool = ctx.enter_context(tc.tile_pool(name="slab32", bufs=3))
        slabh_pool = ctx.enter_context(tc.tile_pool(name="slabh", bufs=2))
        labb_pool = ctx.enter_context(tc.tile_pool(name="labb", bufs=3))
        oh_pool = ctx.enter_context(tc.tile_pool(name="oh", bufs=6))
        psum_pool = ctx.enter_context(tc.tile_pool(name="psum", bufs=1, space="PSUM"))
        out_pool = ctx.enter_context(tc.tile_pool(name="outp", bufs=1))

        iota_t = const_pool.tile([128, K * qb], bf16)
        nc.sync.dma_start(iota_t[:], iota_ext[:])

        psum_full = psum_pool.tile([128, K * qb], f32)
        psum_t = psum_full[: nch * qb, :]

        # PE warmup: ~5us of dense matmuls so the HAM clock gate opens
        # (otherwise every matmul runs at the cold 1.2 GHz rate).
        warm_ps = psum_pool.tile([128, 256], f32)
        if warm:
            for w in range(warm):
                nc.tensor.matmul(
                    warm_ps[:],
                    iota_t[:, :128],
                    iota_t[:, : K * qb],
                    start=(w == 0),
                    stop=(w == warm - 1),
                )

        nblocks = ftot
        blk = 0
        goff = 0
        for fgg in group_sizes:
            gpx = 128 * fgg
            poff = 128 * goff
            slab32 = slab32_pool.tile([128, C * fg], f32, tag="slab32")
            s32 = slab32[:, : C * fgg]
            nc.sync.dma_start(
                s32.rearrange("p (c f) -> p c f", c=C),
                pred_ext[:, poff : poff + gpx].rearrange("c (p f) -> p c f", p=128),
            )
            # slabh layout: [p, (tg, c, b)], col = tg*(nch*qb) + c*qb + b —
            # each tg-group's stationary [128, nch*qb] is a contiguous slice.
            slabh = slabh_pool.tile([128, nch * fg], fp16, tag="slabh")
            slabh_r = slabh[:, : nch * fgg].rearrange(
                "p (tg c b) -> p tg c b", c=nch, b=qb
            )  # [128, fgg/qb, nch, qb]
            slab32_r = s32.rearrange(
                "p (c tg b) -> p tg c b", c=C, b=qb
            )  # in natural (c, t) layout: t = tg*qb + b
            # scaled fp16 cast on ScalarE: out = Copy(in * 2^14)
            nc.scalar.activation(
                slabh_r[:, :, :C, :],
                slab32_r,
                mybir.ActivationFunctionType.Copy,
                scale=PRED_SCALE,
            )
            # ones column via ACT: Copy(0*x + 1) = 1.0 (keeps DVE free)
            nc.scalar.activation(
                slabh_r[:, :, ones_col, :],
                s32[:, :fgg],
                mybir.ActivationFunctionType.Copy,
                bias=1.0,
                scale=0.0,
            )

            # labels arrive as bf16 via SWDGE cast-DMA (no DVE copy needed)
            lbt = labb_pool.tile([128, fg], bf16, tag="labb")
            nc.gpsimd.dma_start(
                lbt[:, :fgg],
                lab_ext[poff : poff + gpx].rearrange("(p f) -> p f", p=128),
            )

            fcg = min(fc, fgg)
            for ci in range(fgg // fcg):
                # oh layout: [p, (tg, j, b)] — each tg-group's moving
                # operand [128, K*qb] is a contiguous slice.
                oh = oh_pool.tile([128, K * fc], fp16, tag="oh")
                oh_r = oh[:, : K * fcg].rearrange(
                    "p (tg j b) -> p tg j b", j=K, b=qb
                )  # [128, fcg/qb, K, qb]
                in0 = (
                    lbt[:, ci * fcg : (ci + 1) * fcg]
                    .rearrange("p (tg b) -> p tg b", b=qb)
                    .unsqueeze(2)
                    .broadcast_to([128, fcg // qb, K, qb])
                )
                in1 = (
                    iota_t[:]
                    .rearrange("p (j b) -> p j b", b=qb)
                    .unsqueeze(1)
                    .broadcast_to([128, fcg // qb, K, qb])
                )
                nc.vector.tensor_tensor(oh_r, in0, in1, mybir.AluOpType.is_equal)
                for tg in range(fcg // qb):
                    tg_abs = (ci * fcg) // qb + tg
                    nc.tensor.matmul(
                        psum_t[:],
                        slabh[:, tg_abs * nch * qb : (tg_abs + 1) * nch * qb],
                        oh[:, tg * K * qb : (tg + 1) * K * qb],
                        start=(blk == 0),
                        stop=(blk == nblocks - qb),
                    )
                    blk += qb
            goff += fgg

        outt = out_pool.tile([128, K * qb], f32)
        nc.vector.memset(outt[:], 0.0)
        nc.vector.tensor_copy(outt[: nch * qb, :], psum_t[:])
        if warm:
            nc.vector.tensor_copy(outt[96:97, :], warm_ps[96:97, : K * qb])
        nc.sync.dma_start(out_ext[:], outt[:])
    nc.compile()
    return nc


def make_iota_np(qb=QB):
    import ml_dtypes

    # value j+1 at [p, j*qb + b]
    v = np.repeat(np.arange(1, K + 1, dtype=np.float32), qb)
    return np.broadcast_to(v, (128, K * qb)).astype(ml_dtypes.bfloat16)


@functools.lru_cache(maxsize=1)
def _get_program():
    return build_nc()


def make_in_maps(pred_flat, labels_flat):
    iota_np = make_iota_np()
    in_maps = []
    for i in range(NCORES):
        sl = slice(i * PCORE, (i + 1) * PCORE)
        in_maps.append(
            {
                "pred": np.ascontiguousarray(pred_flat[:, sl]),
                "labels": np.ascontiguousarray(labels_flat[sl]),
                "iotarep": iota_np,
            }
        )
    return in_maps


def finish_host(parts, num_kernel, qb=QB):
    """parts: per-core [9*qb+1, K*qb] partials. Tiny O(K^2) tail in f64."""
    nch = C + 1
    total = np.sum([p.astype(np.float64) for p in parts], axis=0)
    r = total[: nch * qb, :].reshape(nch, qb, K, qb)
    total = r[:, np.arange(qb), :, np.arange(qb)].sum(axis=0)  # [nch, K]
    S = total[:C, :] / PRED_SCALE  # [8, 32]
    N = total[C, :]  # [32]
    A = N * np.sum(S * S, axis=0)  # [32]
    kk = int(num_kernel)
    A = A[:kk]
    pair = A[:, None] + A[None, :]
    Dm = np.maximum(SIGMA_DIS - np.sqrt(pair), 0.0)
    term = np.log(Dm * Dm + 1.0)
    L = float(np.sum(np.triu(term, k=1)))
    L *= (kk - 1) / kk
    return np.float32(L)


_last_results = None


def kernel(pred_similarities, regions_mask, kernel_labels, num_kernel, **kw):
    global _last_results
    from concourse.bass_utils import run_bass_kernel_spmd

    pred_flat = np.asarray(pred_similarities, dtype=np.float32).reshape(C, PTOT)
    labels_flat = np.asarray(kernel_labels, dtype=np.int32).reshape(PTOT)

    nc = _get_program()
    in_maps = make_in_maps(pred_flat, labels_flat)
    res = run_bass_kernel_spmd(nc, in_maps, list(range(NCORES)))
    _last_results = res
    parts = [res.results[i]["out_s"] for i in range(NCORES)]
    return finish_host(parts, num_kernel)


# revision 32
# speedup vs baseline: 1.0794x; 1.0015x over previous
"""Trainium2 Bass kernel for nn_DiscriminationLoss (segment_reduce).

Measured: ~88-96 us HW exec on 8 cores, rel err ~2e-5 vs the f32
reference (DVE one-hot spine ~70 us + ~20 us NEFF init/drain overhead;
HBM roofline for the 18 MiB/core of traffic is ~52 us).

Strategy (8 NeuronCores, pixel-sharded):
  - Each core gets 1/8 of the 4M pixels: pred slice [8, 524288] f32 and
    labels slice [524288] i32.
  - Pixels are tiled [128 partitions x F free]. For each free column t
    (a "block" of 128 pixels), a one-hot matrix oh[p, j] = (labels[p,t]
    == j+1), j in 0..31 is built on DVE (label 0 = background dropped,
    as in the reference). One-hot generation is batched over FC blocks
    per tensor_tensor(is_equal); all access patterns are dense step-1
    16-bit, which engages the DVE 2x perf mode (~4.4us per 256-block
    chunk — this stream is the kernel's critical path).
  - pred is scaled by 2^14 and cast to fp16 on ScalarE (the scale rides
    the activation's free affine; the ones column for counts is written
    by a second activation with scale=0, bias=1). fp16 keeps ~2^-11
    per-element error; the final loss error lands ~2e-5. Host unscales.
  - The PE contracts QB=8 blocks per matmul (block-diagonal batching to
    amortize the per-instruction floor):
      psum[72, 256] += st[128, 72]^T @ oh[128, 256]
    where st packs 8 blocks' [8 fp16 channels | ones] side by side and
    oh packs their one-hots; only the 8 diagonal [9, 32] sub-blocks of
    the product are meaningful and the host extracts them. PSUM
    accumulates over all 512 matmuls per core. A short warmup burst of
    dense matmuls first trips the PE HAM clock gate to 2.4 GHz
    (otherwise every matmul pays the cold 1.2 GHz rate — measured 2x).
  - Layouts are permuted at the producer side (ACT cast and DVE one-hot
    write through strided APs) so every matmul operand is a contiguous
    single-free-dim slice, which walrus requires.
  - GpSimd only issues the labels' int32->bf16 cast-DMA (SWDGE); it
    must not run streaming SBUF ops: its SBUF port is shared with the
    DVE and any long GpSimd op blocks the one-hot stream (measured
    3-4x stalls on the TT when GpSimd streamed concurrently).
  - Each core emits [128, 256] (PSUM readout + warmup dump row). Host
    sums partials over cores (the "psum" step of the sharding hint) and
    evaluates the tiny O(K^2) pairwise tail in f64.
"""

import sys
import functools

sys.path.insert(0, "/opt/trn_rl_repo")

import numpy as np

C = 8
K = 32
NCORES = 8
H = W = 2048
PTOT = H * W
PCORE = PTOT // NCORES  # 524288
SIGMA_DIS = 3.0
PRED_SCALE = float(2.0**14)

FG = 512   # free-dim length per DMA group (128*FG pixels per group)
FC = 256   # free-dim length per one-hot chunk (blocks per tensor_tensor)
QB = 8     # pixel-blocks batched per matmul (block-diagonal trick)
WARM_MMS = 64  # PE warmup matmuls (trip the HAM clock gate to 2.4 GHz)


def build_nc(pcore=PCORE, fg=FG, fc=FC, qb=QB, warm=WARM_MMS):
    import concourse.bacc as bacc
    import concourse.tile as tile
    import concourse.mybir as mybir
    from contextlib import ExitStack

    ftot = pcore // 128
    assert pcore % 128 == 0
    fg = min(fg, ftot)
    assert ftot % fg == 0 and fg % fc == 0
    # small leading groups prime the pipeline faster
    if ftot >= 8 * fg and fg >= 2 * qb and fg % 2 == 0:
        group_sizes = [fg // 2, fg // 2] + [fg] * (ftot // fg - 1)
    else:
        group_sizes = [fg] * (ftot // fg)
    assert sum(group_sizes) == ftot
    f32 = mybir.dt.float32
    bf16 = mybir.dt.bfloat16
    fp16 = mybir.dt.float16
    i32 = mybir.dt.int32

    nch = C + 1
    ones_col = C
    assert fc % qb == 0

    nc = bacc.Bacc(
        "TRN2", target_bir_lowering=False, debug=False, num_devices=NCORES
    )
    pred_ext = nc.dram_tensor("pred", [C, pcore], f32, kind="ExternalInput")
    lab_ext = nc.dram_tensor("labels", [pcore], i32, kind="ExternalInput")
    iota_ext = nc.dram_tensor("iotarep", [128, K * qb], bf16, kind="ExternalInput")
    # rows 0..nch*qb-1: results; row 96: warmup dump (keeps warm MMs live)
    out_ext = nc.dram_tensor("out_s", [128, K * qb], f32, kind="ExternalOutput")

    with tile.TileContext(nc) as tc, ExitStack() as ctx:
        const_pool = ctx.enter_context(tc.tile_pool(name="const", bufs=1))
        slab32_pool = ctx.enter_context(tc.tile_pool(name="slab32", bufs=3))
        slabh_pool = ctx.enter_context(tc.tile_pool(name="slabh", bufs=2))
        labb_pool = ctx.enter_context(tc.tile_pool(name="labb", bufs=3))
        oh_pool = ctx.enter_context(tc.tile_pool(name="oh", bufs=6))
        psum_pool = ctx.enter_context(tc.tile_pool(name="psum", bufs=1, space="PSUM"))
        out_pool = ctx.enter_context(tc.tile_pool(name="outp", bufs=1))

        iota_t = const_pool.tile([128, K * qb], bf16)
        nc.sync.dma_start(iota_t[:], iota_ext[:])

        psum_full = psum_pool.tile([128, K * qb], f32)
        psum_t = psum_full[: nch * qb, :]

        # PE warmup: ~5us of dense matmuls so the HAM clock gate opens
        # (otherwise every matmul runs at the cold 1.2 GHz rate).
        warm_ps = psum_pool.tile([128, 256], f32)
        if warm:
            for w in range(warm):
                nc.tensor.matmul(
                    warm_ps[:],
                    iota_t[:, :128],
                    iota_t[:, : K * qb],
                    start=(w == 0),
                    stop=(w == warm - 1),
                )

        nblocks = ftot
        blk = 0
        goff = 0
        for fgg in group_sizes:
            gpx = 128 * fgg
            poff = 128 * goff
            slab32 = slab32_pool.tile([128, C * fg], f32, tag="slab32")
            s32 = slab32[:, : C * fgg]
            nc.sync.dma_start(
                s32.rearrange("p (c f) -> p c f", c=C),
                pred_ext[:, poff : poff + gpx].rearrange("c (p f) -> p c f", p=128),
            )
            # slabh layout: [p, (tg, c, b)], col = tg*(nch*qb) + c*qb + b —
            # each tg-group's stationary [128, nch*qb] is a contiguous slice.
            slabh = slabh_pool.tile([128, nch * fg], fp16, tag="slabh")
            slabh_r = slabh[:, : nch * fgg].rearrange(
                "p (tg c b) -> p tg c b", c=nch, b=qb
            )  # [128, fgg/qb, nch, qb]
            slab32_r = s32.rearrange(
                "p (c tg b) -> p tg c b", c=C, b=qb
            )  # in natural (c, t) layout: t = tg*qb + b
            # scaled fp16 cast on ScalarE: out = Copy(in * 2^14)
            nc.scalar.activation(
                slabh_r[:, :, :C, :],
                slab32_r,
                mybir.ActivationFunctionType.Copy,
                scale=PRED_SCALE,
            )
            # ones column via ACT: Copy(0*x + 1) = 1.0 (keeps DVE free)
            nc.scalar.activation(
                slabh_r[:, :, ones_col, :],
                s32[:, :fgg],
                mybir.ActivationFunctionType.Copy,
                bias=1.0,
                scale=0.0,
            )

            # labels arrive as bf16 via SWDGE cast-DMA (no DVE copy needed)
            lbt = labb_pool.tile([128, fg], bf16, tag="labb")
            nc.gpsimd.dma_start(
                lbt[:, :fgg],
                lab_ext[poff : poff + gpx].rearrange("(p f) -> p f", p=128),
            )

            fcg = min(fc, fgg)
            for ci in range(fgg // fcg):
                # oh layout: [p, (tg, j, b)] — each tg-group's moving
                # operand [128, K*qb] is a contiguous slice.
                oh = oh_pool.tile([128, K * fc], fp16, tag="oh")
                oh_r = oh[:, : K * fcg].rearrange(
                    "p (tg j b) -> p tg j b", j=K, b=qb
                )  # [128, fcg/qb, K, qb]
                in0 = (
                    lbt[:, ci * fcg : (ci + 1) * fcg]
                    .rearrange("p (tg b) -> p tg b", b=qb)
                    .unsqueeze(2)
                    .broadcast_to([128, fcg // qb, K, qb])
                )
                in1 = (
                    iota_t[:]
                    .rearrange("p (j b) -> p j b", b=qb)
                    .unsqueeze(1)
                    .broadcast_to([128, fcg // qb, K, qb])
                )
                nc.vector.tensor_tensor(oh_r, in0, in1, mybir.AluOpType.is_equal)
                for tg in range(fcg // qb):
                    tg_abs = (ci * fcg) // qb + tg
                    nc.tensor.matmul(
                        psum_t[:],
                        slabh[:, tg_abs * nch * qb : (tg_abs + 1) * nch * qb],
                        oh[:, tg * K * qb : (tg + 1) * K * qb],
                        start=(blk == 0),
                        stop=(blk == nblocks - qb),
                    )
                    blk += qb
            goff += fgg

        outt = out_pool.tile([128, K * qb], f32)
        nc.vector.memset(outt[:], 0.0)
        nc.vector.tensor_copy(outt[: nch * qb, :], psum_t[:])
        if warm:
            nc.vector.tensor_copy(outt[96:97, :], warm_ps[96:97, : K * qb])
        nc.sync.dma_start(out_ext[:], outt[:])
    nc.compile()
    return nc


def make_iota_np(qb=QB):
    import ml_dtypes

    # value j+1 at [p, j*qb + b]
    v = np.repeat(np.arange(1, K + 1, dtype=np.float32), qb)
    return np.broadcast_to(v, (128, K * qb)).astype(ml_dtypes.bfloat16)


@functools.lru_cache(maxsize=1)
def _get_program():
    return build_nc()


def make_in_maps(pred_flat, labels_flat):
    iota_np = make_iota_np()
    in_maps = []
    for i in range(NCORES):
        sl = slice(i * PCORE, (i + 1) * PCORE)
        in_maps.append(
            {
                "pred": np.ascontiguousarray(pred_flat[:, sl]),
                "labels": np.ascontiguousarray(labels_flat[sl]),
                "iotarep": iota_np,
            }
        )
    return in_maps


def finish_host(parts, num_kernel, qb=QB):
    """parts: per-core [9*qb+1, K*qb] partials. Tiny O(K^2) tail in f64."""
    nch = C + 1
    total = np.sum([p.astype(np.float64) for p in parts], axis=0)
    r = total[: nch * qb, :].reshape(nch, qb, K, qb)
    total = r[:, np.arange(qb), :, np.arange(qb)].sum(axis=0)  # [nch, K]
    S = total[:C, :] / PRED_SCALE  # [8, 32]
    N = total[C, :]  # [32]
    A = N * np.sum(S * S, axis=0)  # [32]
    kk = int(num_kernel)
    A = A[:kk]
    pair = A[:, None] + A[None, :]
    Dm = np.maximum(SIGMA_DIS - np.sqrt(pair), 0.0)
    term = np.log(Dm * Dm + 1.0)
    L = float(np.sum(np.triu(term, k=1)))
    L *= (kk - 1) / kk
    return np.float32(L)


_last_results = None


def kernel(pred_similarities, regions_mask, kernel_labels, num_kernel, **kw):
    global _last_results
    from concourse.bass_utils import run_bass_kernel_spmd

    pred_flat = np.asarray(pred_similarities, dtype=np.float32).reshape(C, PTOT)
    labels_flat = np.asarray(kernel_labels, dtype=np.int32).reshape(PTOT)

    nc = _get_program()
    in_maps = make_in_maps(pred_flat, labels_flat)
    res = run_bass_kernel_spmd(nc, in_maps, list(range(NCORES)))
    _last_results = res
    parts = [res.results[i]["out_s"] for i in range(NCORES)]
    return finish_host(parts, num_kernel)
